# revision 14
# baseline (speedup 1.0000x reference)
"""Trainium2 Bass kernel for nn_NumDualDescriptorAB.

Reference computation:
    agg[b,w]   = mean(seq[b, w:w+8, :], axis=0)          (sliding window, Nw = S-7)
    y[b,w]     = agg[b,w] @ M.T
    Nk[w]      = Acoeff[:, w%L] * Bbasis[w%L, :]
    D          = mean((y - Nk)^2)

Algebraic decomposition (everything heavy becomes matmuls with tiny outputs):
    count = B*Nw*m
    t1 = sum_{b,w} agg MtM agg^T = <M^T M, G>_F   with G = sum agg^T agg   (m x m)
    t2 = sum_{b,w} y . Nk = sum_{b,s} seq[b,s] . P[s]    with P = W^T (Nk M)  (S x m)
    t3 = B * ||Nk||^2
    D  = (t1 - 2 t2 + t3) / count

Device (8 cores, data-parallel over batch; 4 batches/core) computes G and
X^T = sum_chunks P_chunk^T seq_chunk per core; the tiny combination happens
on host in float64.

The sliding-window aggregation itself runs on the TensorEngine via a banded
constant matrix W (lhsT), chunked 121 windows at a time so each chunk's
windows only need the chunk's own 128 rows.
"""

import os

# The device run goes through jax's axon/neuron backend; a cpu-only pin
# (used for reference computations elsewhere) would hide the NeuronCores.
if os.environ.get("JAX_PLATFORMS", "").strip() == "cpu":
    del os.environ["JAX_PLATFORMS"]

import numpy as np
import ml_dtypes

B, S, m, L, RANK = 32, 2048, 128, 64, 8
Nw = S - RANK + 1  # 2041
NCORES = 8
BPC = B // NCORES  # batches per core = 4
CH = 121  # windows per chunk (window w needs rows w..w+7, so 121+7=128 rows)
NCH = (Nw + CH - 1) // CH  # 17 chunks
TAILW = Nw - (NCH - 1) * CH  # 105 windows in the last chunk
CW = BPC * m  # free columns per chunk = 512

BF16 = ml_dtypes.bfloat16

_NC_CACHE = {}

N_WARMUP_MM = 2  # dummy N=256 matmuls to warm the PE HAM clock gate


def _build_nc():
    import concourse.bacc as bacc
    import concourse.mybir as mybir
    import concourse.tile as tile

    bf = mybir.dt.bfloat16
    f32 = mybir.dt.float32
    f8 = mybir.dt.float8e4

    nc = bacc.Bacc("TRN2", target_bir_lowering=False, debug=False,
                   enable_partition_id=False)

    seq_d = nc.dram_tensor("seq", [128, NCH * CW], bf, kind="ExternalInput")
    w_d = nc.dram_tensor("wmat", [128, 2 * m], bf, kind="ExternalInput")
    p_d = nc.dram_tensor("pmat", [128, NCH * m], bf, kind="ExternalInput")
    out_d = nc.dram_tensor("out", [128, m + CW], f32, kind="ExternalOutput")

    # chunk pairs per PSUM round (17 chunks -> 8 pairs + 1 singleton)
    PAIRS = [(c, min(c + 2, NCH)) for c in range(0, NCH, 2)]
    NP = len(PAIRS)
    # seq DMA pieces (pair-aligned), in consumption order on the sync ring
    PIECES = [(0, 2), (2, 4), (4, 8), (8, 12), (12, 16), (16, NCH)]

    with tile.TileContext(nc) as tc:
        with (
            tc.tile_pool(name="const", bufs=1) as cpool,
            tc.tile_pool(name="agg", bufs=2) as apool,
            tc.tile_pool(name="psa", bufs=2, space="PSUM") as pspool,
            tc.tile_pool(name="psw", bufs=1, space="PSUM") as pswarm,
            tc.tile_pool(name="psacc", bufs=1, space="PSUM") as accpool,
        ):
            # One HWDGE ring (sync), FIFO in consumption order:
            # wmat, seq pieces 0-1, pmat (needed from X-mm of pair 1 on),
            # then the remaining seq pieces.
            s_w = cpool.tile([128, 2 * m], bf, tag="w")
            nc.sync.dma_start(out=s_w[:], in_=w_d[:])
            s_p = cpool.tile([128, NCH * m], bf, tag="p")
            seq_tiles = [None] * NCH  # chunk -> (tile, base col)

            def load_piece(pc, eng):
                a, b_ = PIECES[pc]
                t = cpool.tile([128, (b_ - a) * CW], bf, tag=f"seq{a}")
                eng.dma_start(out=t[:], in_=seq_d[:, a * CW:b_ * CW])
                for c in range(a, b_):
                    seq_tiles[c] = (t, (c - a) * CW)

            # two independent descriptor generators draining concurrently:
            # even pieces on the sync HWDGE ring, odd pieces via gpsimd SWDGE
            load_piece(0, nc.sync)
            load_piece(1, nc.gpsimd)
            nc.sync.dma_start(out=s_p[:], in_=p_d[:])
            for pc in range(2, len(PIECES)):
                load_piece(pc, nc.sync if pc % 2 == 0 else nc.gpsimd)

            def seq_ap(c):
                t, o = seq_tiles[c]
                return t[:, o:o + CW]

            # PE warmup while the first seq piece is in flight
            if N_WARMUP_MM:
                warm_ps = pswarm.tile([128, 2 * m], f32, tag="warm")
                for _ in range(N_WARMUP_MM):
                    nc.tensor.matmul(warm_ps[:], s_w[:, 0:m], s_w[:],
                                     start=True, stop=True)

            G_ps = accpool.tile([128, m], f32, tag="G")
            X_ps = accpool.tile([128, CW], f32, tag="X")

            agg_tiles = {}

            def emit_win(p):
                c0, c1 = PAIRS[p]
                agg_ps = pspool.tile([128, 2 * CW], f32, tag="aggps")
                agg_tiles[p] = agg_ps
                for k, c in enumerate(range(c0, c1)):
                    wsel = s_w[:, 0:m] if c < NCH - 1 else s_w[:, m:2 * m]
                    nc.tensor.matmul(agg_ps[:, k * CW:(k + 1) * CW], wsel,
                                     seq_ap(c), start=True, stop=True)

            def emit_x(p):
                for c in range(*PAIRS[p]):
                    nc.tensor.matmul(
                        X_ps[:], s_p[:, c * m:(c + 1) * m], seq_ap(c),
                        start=(c == 0), stop=(c == NCH - 1),
                        skip_group_check=True,
                    )

            # software pipeline: CAST(p) overlaps PE's win(p+1)/X(p)
            emit_win(0)
            for p in range(NP):
                c0, c1 = PAIRS[p]
                n = c1 - c0
                aggb = apool.tile([128, 2 * CW], bf, tag="aggb")
                if n == 2:
                    # halves live in different PSUM banks -> DVE || ACT
                    nc.vector.tensor_copy(aggb[:, :CW], agg_tiles[p][:, :CW])
                    nc.scalar.copy(aggb[:, CW:2 * CW], agg_tiles[p][:, CW:2 * CW])
                else:
                    nc.vector.tensor_copy(aggb[:, :n * CW], agg_tiles[p][:, :n * CW])
                if p + 1 < NP:
                    emit_win(p + 1)
                emit_x(p)
                for j in range(n * BPC):
                    blk = aggb[:, j * m:(j + 1) * m]
                    nc.tensor.matmul(
                        G_ps[:], blk, blk,
                        start=(p == 0 and j == 0),
                        stop=(p == NP - 1 and j == n * BPC - 1),
                        skip_group_check=True,
                    )

            # X finishes before the last grams: copy + DMA it out early
            s_out = cpool.tile([128, m + CW], f32, tag="out")
            nc.scalar.copy(s_out[:, m:m + CW], X_ps[:])
            nc.sync.dma_start(out=out_d[:, m:m + CW], in_=s_out[:, m:m + CW])
            nc.vector.tensor_copy(s_out[:, 0:m], G_ps[:])
            nc.scalar.dma_start(out=out_d[:, 0:m], in_=s_out[:, 0:m])

    nc.compile()
    return nc


def get_nc():
    if "nc" not in _NC_CACHE:
        _NC_CACHE["nc"] = _build_nc()
    return _NC_CACHE["nc"]


def _chunk_rows():
    rows = CH * np.arange(NCH)[:, None] + np.arange(128)[None, :]  # [NCH, 128]
    valid = rows < S
    return rows, valid


def host_prep(seq_batch, M, Acoeff, Bbasis):
    """Build per-core device inputs + host-side exact terms."""
    rows, valid = _chunk_rows()
    rows_c = np.minimum(rows, S - 1)

    # seq image: per core [128, NCH, BPC, m] with seq_img[p, c, j] = seq[4k+j, 121c+p]
    g = seq_batch[:, rows_c, :].astype(BF16)  # [B, NCH, 128, m]
    g[:, ~valid, :] = 0
    imgs = np.ascontiguousarray(
        g.reshape(NCORES, BPC, NCH, 128, m).transpose(0, 3, 2, 1, 4)
    ).reshape(NCORES, 128, NCH * BPC * m)

    # banded window matrices (lhsT): out[w, n] = sum_k W[k, w] rhs[k, n]
    k = np.arange(128)[:, None]
    w = np.arange(128)[None, :]
    band = ((k - w >= 0) & (k - w < RANK)).astype(np.float32) / RANK
    wmain = band * (w < CH)
    wtail = band * (w < TAILW)
    wmat = np.concatenate([wmain, wtail], axis=1).astype(BF16)  # [128, 256]

    # Nk / Ntil / P in float64
    M64 = np.asarray(M, np.float64)
    kmod = np.arange(Nw) % L
    Nk = (np.asarray(Acoeff, np.float64).T[kmod]
          * np.asarray(Bbasis, np.float64)[kmod])  # [Nw, m]
    Ntil = Nk @ M64  # [Nw, m]
    csum = np.concatenate([np.zeros((1, m)), np.cumsum(Ntil, axis=0)])
    s = np.arange(S)
    lo = np.maximum(s - (RANK - 1), 0)
    hi = np.minimum(s, Nw - 1)
    P = (csum[hi + 1] - csum[lo]) / RANK  # [S, m]

    pr = P[rows_c].astype(np.float32)  # [NCH, 128, m]
    pvalid = valid & (np.arange(128) < CH)[None, :]
    pr[~pvalid] = 0
    pmat = np.ascontiguousarray(pr.transpose(1, 0, 2)).reshape(128, NCH * m)
    pmat = pmat.astype(BF16)

    t3 = B * float((Nk ** 2).sum())
    MtM = M64.T @ M64
    return imgs, wmat, pmat, MtM, t3


def combine(results, MtM, t3):
    """results: list of 8 arrays [128, 640] f32 -> scalar D."""
    G = np.zeros((m, m), np.float64)
    t2 = 0.0
    for r in results:
        r = np.asarray(r, np.float64)
        G += r[:, :m]
        for j in range(BPC):
            t2 += np.trace(r[:, m + j * m:m + (j + 1) * m])
    t1 = float((MtM * G).sum())
    D = (t1 - 2.0 * t2 + t3) / (B * Nw * m)
    return np.float32(D)


def kernel(seq_batch, M, Acoeff, Bbasis):
    from concourse.bass_utils import run_bass_kernel_spmd

    seq_batch = np.asarray(seq_batch, np.float32)
    imgs, wmat, pmat, MtM, t3 = host_prep(seq_batch, M, Acoeff, Bbasis)

    nc = get_nc()
    in_maps = [
        {"seq": imgs[c], "wmat": wmat, "pmat": pmat} for c in range(NCORES)
    ]
    res = run_bass_kernel_spmd(nc, in_maps, core_ids=list(range(NCORES)))
    outs = [res.results[c]["out"] for c in range(NCORES)]
    return combine(outs, MtM, t3)
